# revision 5
# baseline (speedup 1.0000x reference)
"""TransformerConv (heads=1) + ELU — Bass/Tile kernel v4 on 8 NeuronCores.

Sharding: 1D partition by target node (12500 dst/core), halo exchange on host.

v4 layout: lane = dst. Each dst node owns one (block, lane) slot; its edges
occupy columns 0..deg-1 of that lane. Blocks are 128 dsts, degree-sorted so
per-block column counts C_b are tight. Blocks are grouped into gangs of
<=GB_MAX blocks / ~COLBUDGET columns for DMA + wide DVE ops.

Device math per block b (C = C_b cols):
  qk = x@Wqk + bqk   (phase 1, [dst, d])        Wqk = scale*Wq@Wk^T folded
  skipT = Ws^T@x^T + bs + bv*ind (phase 1, [d, dst] transposed, const lhsT)
  prod[p,c,d] = qk[p,d] * xg[p,c,d]             (DVE TT, qk bcast over c)
  logit[p,c] = sum_d prod                       (DVE reduce X, per gang)
  ex = exp(logit)                               (ACT, per gang)
  exm[p,c,2] = ex * maskd                       (DVE, pair-expanded)
  den[p] = sum_c exm ; rec = 1/(den+eps)        (DVE, per block/gang)
  exd = exm * rec                               (DVE tensor_scalar, alpha/2)
  wx[p,c,d] = exd[p,c] * xg[p,c,d]              (DVE TT, pair-trick 2x)
  pagT[d,p] += wx[:,c,:]^T (lhsT=wx, rhs=ident) (PE, PSUM accumulate)
  aggvT = (2Wv)^T @ pagT                        (PE, const lhsT, per gang)
  zT = aggvT + skipT ; out = ELU(zT)            (GPSIMD/ACT/DVE, per gang)
Output is produced transposed [d, dst]; host transposes back.
"""
import math
import numpy as np
import ml_dtypes

BF16 = ml_dtypes.bfloat16

N, E, D = 100000, 800000, 128
M_CORES = 8
DPC = N // M_CORES                 # 12500
NB = (DPC + 127) // 128            # 98
DST_PAD = NB * 128                 # 12544
SCALE = 1.0 / math.sqrt(D)
COLBUDGET = 48                     # target columns per gang
GB_MAX = 8                         # max blocks per gang (PSUM bank cap)


def _schedule(edge_index):
    """Shared (cross-core) block/gang schedule + per-core orderings."""
    src = np.asarray(edge_index[0], dtype=np.int64)
    dst = np.asarray(edge_index[1], dtype=np.int64)
    core = dst // DPC
    ld = dst - core * DPC

    percore = []
    md = np.zeros((M_CORES, NB), np.int64)
    for c in range(M_CORES):
        sel = core == c
        deg = np.bincount(ld[sel], minlength=DPC)[:DPC]
        order = np.argsort(-deg, kind="stable")
        ds = deg[order]
        padded = np.zeros(DST_PAD, np.int64)
        padded[:DPC] = ds
        md[c] = padded[::128]
        percore.append((sel, deg, order))
    C_blk = np.maximum(md.max(axis=0), 1)          # [NB]

    # gangs: consecutive blocks, <=GB_MAX blocks, ~COLBUDGET cols
    gangs = []                                     # list of (b0, nb)
    b = 0
    while b < NB:
        nb, cols = 1, C_blk[b]
        while (b + nb < NB and nb < GB_MAX
               and cols + C_blk[b + nb] <= COLBUDGET + 8):
            cols += C_blk[b + nb]
            nb += 1
        gangs.append((b, nb))
        b += nb
    colbase = np.concatenate([[0], np.cumsum(C_blk)]).astype(np.int64)
    TOTC = int(colbase[-1])
    return percore, src, ld, C_blk, gangs, colbase, TOTC


def _host_prep(edge_index, xb):
    percore, src, ld, C_blk, gangs, colbase, TOTC = _schedule(edge_index)
    cores = []
    for c in range(M_CORES):
        sel, deg, order = percore[c]
        rank_of = np.empty(DPC, np.int64)
        rank_of[order] = np.arange(DPC)

        e_ld = ld[sel]
        e_src = src[sel]
        r = rank_of[e_ld]
        o2 = np.argsort(r, kind="stable")
        rs = r[o2]
        srcs = e_src[o2]
        first = np.searchsorted(rs, rs)
        cc = np.arange(len(rs)) - first            # within-dst edge rank
        b = rs // 128
        lane = rs % 128
        col = colbase[b] + cc
        assert (cc < C_blk[b]).all()

        xg = np.zeros((128, TOTC, 128), BF16)
        xg[lane, col] = xb[srcs]
        maskb = np.full((128, TOTC), -40.0, BF16)
        maskb[lane, col] = 0.0

        xs = np.zeros((DST_PAD, 128), BF16)
        xs[:DPC] = xb[c * DPC + order]
        xTs = np.ascontiguousarray(xs.T)
        ind = np.zeros((1, DST_PAD), BF16)
        ind[0, :DPC] = (deg[order] > 0).astype(BF16)

        cores.append({"xg": xg.reshape(128, -1), "maskb": maskb,
                      "xTs": xTs, "ind": ind, "order": order})
    return cores, C_blk, gangs, colbase, TOTC


def _build_nc(C_blk, gangs, colbase, TOTC):
    from contextlib import ExitStack
    import concourse.tile as tile
    from concourse import bacc, mybir

    fp32 = mybir.dt.float32
    bf16 = mybir.dt.bfloat16
    Alu = mybir.AluOpType
    Act = mybir.ActivationFunctionType

    nc = bacc.Bacc("TRN2", target_bir_lowering=False, debug=False)

    xg_d = nc.dram_tensor("xg", [128, TOTC * 128], bf16, kind="ExternalInput").ap()
    mk_d = nc.dram_tensor("maskb", [128, TOTC], bf16, kind="ExternalInput").ap()
    xTs_d = nc.dram_tensor("xTs", [128, DST_PAD], bf16, kind="ExternalInput").ap()
    ind_d = nc.dram_tensor("ind", [1, DST_PAD], bf16, kind="ExternalInput").ap()
    Wqk_d = nc.dram_tensor("Wqk", [128, 128], bf16, kind="ExternalInput").ap()
    Ws_d = nc.dram_tensor("Ws", [128, 128], bf16, kind="ExternalInput").ap()
    Wv2_d = nc.dram_tensor("Wv2", [128, 128], bf16, kind="ExternalInput").ap()
    bqk_d = nc.dram_tensor("bqk1", [1, 128], bf16, kind="ExternalInput").ap()
    bs_d = nc.dram_tensor("bs1", [1, 128], bf16, kind="ExternalInput").ap()
    bsc_d = nc.dram_tensor("bsc", [128, 1], bf16, kind="ExternalInput").ap()
    bv_d = nc.dram_tensor("bv1", [1, 128], bf16, kind="ExternalInput").ap()
    ident_d = nc.dram_tensor("ident", [128, 128], bf16, kind="ExternalInput").ap()
    out_d = nc.dram_tensor("outT", [128, DST_PAD], bf16, kind="ExternalOutput").ap()

    with tile.TileContext(nc) as tc, ExitStack() as ctx:
        const_p = ctx.enter_context(tc.tile_pool(name="const", bufs=1))

        w_qk = const_p.tile([128, 128], bf16, tag="wqk")
        w_s = const_p.tile([128, 128], bf16, tag="ws")
        w_v2 = const_p.tile([128, 128], bf16, tag="wv2")
        b_qk = const_p.tile([1, 128], bf16, tag="bqk")
        b_s = const_p.tile([1, 128], bf16, tag="bs")
        b_v = const_p.tile([1, 128], bf16, tag="bv")
        ident = const_p.tile([128, 128], bf16, tag="ident")
        nc.sync.dma_start(w_qk[:], Wqk_d[:])
        nc.sync.dma_start(w_s[:], Ws_d[:])
        nc.sync.dma_start(w_v2[:], Wv2_d[:])
        nc.sync.dma_start(b_qk[:], bqk_d[:])
        nc.sync.dma_start(b_s[:], bs_d[:])
        nc.sync.dma_start(b_v[:], bv_d[:])
        nc.sync.dma_start(ident[:], ident_d[:])
        ones1 = const_p.tile([1, 128], bf16, tag="ones1")
        nc.vector.memset(ones1[:], 1.0)
        negone = const_p.tile([128, 1], fp32, tag="negone")
        nc.vector.memset(negone[:], -1.0)
        bs_col = const_p.tile([128, 1], bf16, tag="bscol")
        nc.sync.dma_start(bs_col[:], bsc_d[:])

        # ---------------- phase 2: per-gang edge attention ----------------
        with tc.tile_pool(name="gxg", bufs=3) as gxg_p, \
             tc.tile_pool(name="gmk", bufs=2) as gmk_p, \
             tc.tile_pool(name="p1x", bufs=3) as p1x, \
             tc.tile_pool(name="pw", bufs=3) as pw_p, \
             tc.tile_pool(name="h1p", bufs=2) as h1_p, \
             tc.tile_pool(name="smal", bufs=3) as small_p, \
             tc.tile_pool(name="zp", bufs=2) as z_p, \
             tc.tile_pool(name="op", bufs=2) as o_p, \
             tc.tile_pool(name="p1ps", bufs=2, space="PSUM") as p1ps, \
             tc.tile_pool(name="pagps", bufs=2, space="PSUM") as pag_ps, \
             tc.tile_pool(name="aggps", bufs=2, space="PSUM") as agg_ps:
            CMAX = max(int(sum(C_blk[b0:b0 + nb])) for (b0, nb) in gangs)
            import os as _os
            LOGIT_DT = (mybir.dt.float32 if _os.environ.get("V4_FP32LOGIT")
                        else mybir.dt.float16)
            def _emit_tail(zT, e, b0, nb):
                # ELU: o = min(max(z,0), exp(z)-1); e was emitted with the body
                em = o_p.tile([128, GB_MAX * 128], bf16, tag="em", name="em")
                nc.vector.tensor_scalar_add(em[:, 0:nb * 128],
                                            e[:, 0:nb * 128], -1.0)
                o = o_p.tile([128, GB_MAX * 128], bf16, tag="o", name="o")
                nc.vector.scalar_tensor_tensor(
                    out=o[:, 0:nb * 128], in0=zT[:, 0:nb * 128], scalar=0.0,
                    in1=em[:, 0:nb * 128], op0=Alu.max, op1=Alu.min)
                nc.sync.dma_start(out_d[:, b0 * 128:(b0 + nb) * 128],
                                  o[:, 0:nb * 128])

            pending = None
            gi = 0
            for (b0, nb) in gangs:
                gi += 1
                cb = int(colbase[b0])
                SC = int(sum(C_blk[b0:b0 + nb]))     # gang columns
                # --- fused phase 1 for this gang's nb blocks ---
                gw = nb * 128
                xt = p1x.tile([128, GB_MAX * 128], bf16, tag="xst")
                nc.sync.dma_start(xt[:, 0:gw],
                                  xTs_d[:, b0 * 128:b0 * 128 + gw])
                indt = p1x.tile([1, GB_MAX * 128], bf16, tag="indt")
                nc.sync.dma_start(indt[:, 0:gw],
                                  ind_d[0:1, b0 * 128:b0 * 128 + gw])
                qk_g = const_p.tile([128, gw], bf16, tag=f"qk{gi}",
                                    name=f"qkg{gi}")
                skipT_g = const_p.tile([128, gw], bf16, tag=f"sk{gi}",
                                       name=f"skg{gi}")
                for h0 in range(0, nb, 4):
                    hn = min(4, nb - h0)
                    pq = p1ps.tile([128, 4, 128], fp32, tag="pq")
                    pst = p1ps.tile([128, 4, 128], fp32, tag="pst")
                    for i in range(hn):
                        jj = h0 + i
                        xchunk = xt[:, jj * 128:(jj + 1) * 128]
                        ichunk = indt[0:1, jj * 128:(jj + 1) * 128]
                        nc.tensor.matmul(out=pq[:, i, :], lhsT=xchunk,
                                         rhs=w_qk[:], start=True, stop=False)
                        nc.tensor.matmul(out=pq[:, i, :], lhsT=ones1[:],
                                         rhs=b_qk[:], start=False, stop=True)
                        nc.tensor.matmul(out=pst[:, i, :], lhsT=w_s[:],
                                         rhs=xchunk, start=True, stop=False)
                        nc.tensor.matmul(out=pst[:, i, :], lhsT=b_v[:],
                                         rhs=ichunk, start=False, stop=True)
                    nc.scalar.activation(
                        qk_g[:, h0 * 128:(h0 + hn) * 128]
                        .rearrange("p (j e) -> p j e", e=128),
                        pq[:, 0:hn, :], Act.Copy)
                    nc.scalar.activation(
                        skipT_g[:, h0 * 128:(h0 + hn) * 128]
                        .rearrange("p (j e) -> p j e", e=128),
                        pst[:, 0:hn, :], Act.Identity, bias=bs_col[:, 0:1])
                xgt = gxg_p.tile([128, CMAX, 128], bf16, tag="xg")
                nc.sync.dma_start(
                    xgt[:, 0:SC, :], xg_d[:, cb * 128:(cb + SC) * 128]
                    .rearrange("p (s e) -> p s e", e=128))
                mkt = gmk_p.tile([128, CMAX], bf16, tag="mk")
                nc.sync.dma_start(mkt[:, 0:SC], mk_d[:, cb:cb + SC])

                prod = pw_p.tile([128, CMAX, 128], bf16, tag="pw")
                lb = 0
                for j in range(nb):
                    b = b0 + j
                    C = int(C_blk[b])
                    qkb = qk_g[:, j * 128:(j + 1) * 128] \
                        .rearrange("p (one d) -> p one d", one=1) \
                        .to_broadcast([128, C, 128])
                    nc.vector.tensor_tensor(
                        out=prod[:, lb:lb + C, :], in0=xgt[:, lb:lb + C, :],
                        in1=qkb, op=Alu.mult)
                    lb += C

                # two 2x TT folds halve the 1x reduce's input: 128 -> 32 wide
                h1 = h1_p.tile([128, CMAX, 64], bf16, tag="h1")
                nc.vector.tensor_tensor(
                    out=h1[:, 0:SC, :], in0=prod[:, 0:SC, 0:64],
                    in1=prod[:, 0:SC, 64:128], op=Alu.add)
                h2 = h1_p.tile([128, CMAX, 32], bf16, tag="h2")
                nc.vector.tensor_tensor(
                    out=h2[:, 0:SC, :], in0=h1[:, 0:SC, 0:32],
                    in1=h1[:, 0:SC, 32:64], op=Alu.add)
                logit = small_p.tile([128, CMAX], LOGIT_DT, tag="logit")
                with nc.allow_low_precision(reason="fp16 logits, |l|<30"):
                    nc.vector.tensor_reduce(
                        out=logit[:, 0:SC], in_=h2[:, 0:SC, :], op=Alu.add,
                        axis=mybir.AxisListType.X)
                logitm = small_p.tile([128, CMAX], LOGIT_DT, tag="logitm")
                nc.vector.tensor_tensor(
                    out=logitm[:, 0:SC], in0=logit[:, 0:SC], in1=mkt[:, 0:SC],
                    op=Alu.add)
                # exp -> pair layout, den accumulated by ACT per block
                expair = small_p.tile([128, CMAX, 2], bf16, tag="expair")
                den = small_p.tile([128, GB_MAX], fp32, tag="den")
                lb = 0
                for j in range(nb):
                    C = int(C_blk[b0 + j])
                    nc.scalar.activation(
                        expair[:, lb:lb + C, :],
                        logitm[:, lb:lb + C]
                        .rearrange("p (c one) -> p c one", one=1)
                        .to_broadcast([128, C, 2]),
                        Act.Exp, accum_out=den[:, j:j + 1])
                    lb += C
                dene = small_p.tile([128, GB_MAX], fp32, tag="dene")
                nc.vector.tensor_scalar_add(dene[:, 0:nb], den[:, 0:nb], 1e-30)
                rec = small_p.tile([128, GB_MAX], fp32, tag="rec")
                nc.vector.reciprocal(rec[:, 0:nb], dene[:, 0:nb])

                wx = prod      # reuse the prod tile; reduce->rec chain orders it
                exd = small_p.tile([128, CMAX, 2], bf16, tag="exd")
                zT = z_p.tile([128, GB_MAX * 128], bf16, tag="zT")
                lb = 0
                for j in range(nb):
                    C = int(C_blk[b0 + j])
                    nc.scalar.activation(exd[:, lb:lb + C, :],
                                         expair[:, lb:lb + C, :],
                                         Act.Copy, scale=rec[:, j:j + 1])
                    lb += C
                lb = 0
                for h0 in range(0, nb, 4):
                    hn = min(4, nb - h0)
                    hw = hn * 128
                    pagT = pag_ps.tile([128, 4, 128], fp32, tag="pagT")
                    for i in range(hn):
                        j = h0 + i
                        b = b0 + j
                        C = int(C_blk[b])
                        e4 = exd[:, lb:lb + C, :] \
                            .rearrange("p c (one t) -> p c one t", one=1) \
                            .to_broadcast([128, C, 64, 2])
                        nc.vector.tensor_tensor(
                            out=wx[:, lb:lb + C, :]
                            .rearrange("p c (r t) -> p c r t", t=2),
                            in0=xgt[:, lb:lb + C, :]
                            .rearrange("p c (r t) -> p c r t", t=2),
                            in1=e4, op=Alu.mult)
                        for ci in range(C):
                            nc.tensor.matmul(out=pagT[:, i, :],
                                             lhsT=wx[:, lb + ci, :],
                                             rhs=ident[:],
                                             start=(ci == 0), stop=(ci == C - 1))
                        lb += C
                    pagT_sb = z_p.tile([128, 4, 128], bf16, tag="pagTsb")
                    nc.scalar.activation(pagT_sb[:, 0:hn, :], pagT[:, 0:hn, :],
                                         Act.Copy)
                    aggvT = agg_ps.tile([128, 512], fp32, tag="aggvT")
                    nc.tensor.matmul(
                        out=aggvT[:, 0:hw], lhsT=w_v2[:],
                        rhs=pagT_sb[:, 0:hn, :].rearrange("p j e -> p (j e)"),
                        start=True, stop=False)
                    nc.tensor.matmul(
                        out=aggvT[:, 0:hw], lhsT=ident[:],
                        rhs=skipT_g[:, h0 * 128:h0 * 128 + hw],
                        start=False, stop=True)
                    nc.scalar.activation(zT[:, h0 * 128:h0 * 128 + hw],
                                         aggvT[:, 0:hw], Act.Copy)
                # defer the ELU tail by one gang (avoids DVE head-of-line
                # stall on ACT's exp): emit previous gang's tail now
                e = o_p.tile([128, GB_MAX * 128], bf16, tag="e", name="e")
                nc.scalar.activation(e[:, 0:nb * 128], zT[:, 0:nb * 128], Act.Exp)
                if pending is not None:
                    _emit_tail(*pending)
                pending = (zT, e, b0, nb)
            if pending is not None:
                _emit_tail(*pending)

    nc.compile()
    return nc


_NC_CACHE = {}


def _get_nc(C_blk, gangs, colbase, TOTC):
    key = (tuple(C_blk), tuple(gangs))
    if key not in _NC_CACHE:
        _NC_CACHE.clear()
        _NC_CACHE[key] = _build_nc(C_blk, gangs, colbase, TOTC)
    return _NC_CACHE[key]


def _make_in_maps(inputs, cores):
    wq = np.asarray(inputs["Wq"], np.float32)
    wk = np.asarray(inputs["Wk"], np.float32)
    Wqk = (SCALE * (wq @ wk.T)).astype(BF16)
    bqk = (SCALE * (np.asarray(inputs["bq"], np.float32) @ wk.T)).astype(BF16)
    ws = np.asarray(inputs["Ws"], np.float32).astype(BF16)
    # ACT-accumulated den counts each exp twice (pair layout) -> fold 2x here
    wv2 = (2.0 * np.asarray(inputs["Wv"], np.float32)).astype(BF16)
    bs = np.asarray(inputs["bs"], np.float32).astype(BF16)
    bv = np.asarray(inputs["bv"], np.float32).astype(BF16)
    ident = np.eye(128, dtype=np.float32).astype(BF16)

    in_maps = []
    for c in range(M_CORES):
        co = cores[c]
        in_maps.append({
            "xg": co["xg"], "maskb": co["maskb"], "xTs": co["xTs"],
            "ind": co["ind"],
            "Wqk": Wqk, "Ws": ws, "Wv2": wv2,
            "bqk1": bqk.reshape(1, 128), "bs1": bs.reshape(1, 128),
            "bsc": bs.reshape(128, 1),
            "bv1": bv.reshape(1, 128), "ident": ident,
        })
    return in_maps


def _prepare(x, edge_index, Wq, bq, Wk, bk, Wv, bv, Ws, bs):
    xb = np.asarray(x, np.float32).astype(BF16)
    cores, C_blk, gangs, colbase, TOTC = _host_prep(edge_index, xb)
    in_maps = _make_in_maps(
        {"Wq": Wq, "Wk": Wk, "Wv": Wv, "Ws": Ws,
         "bq": bq, "bs": bs, "bv": bv}, cores)
    nc = _get_nc(C_blk, gangs, colbase, TOTC)
    return nc, in_maps, cores


def kernel(x, edge_index, Wq, bq, Wk, bk, Wv, bv, Ws, bs):
    from concourse import bass_utils

    nc, in_maps, cores = _prepare(x, edge_index, Wq, bq, Wk, bk, Wv, bv, Ws, bs)
    res = bass_utils.run_bass_kernel_spmd(nc, in_maps, core_ids=list(range(M_CORES)))
    out = np.zeros((N, 128), np.float32)
    for c in range(M_CORES):
        outT = res.results[c]["outT"].astype(np.float32)   # [128, DST_PAD]
        rows = outT.T                                      # [DST_PAD, 128]
        order = cores[c]["order"]
        out[c * DPC + order] = rows[:DPC]
    return out


def trace_run(inputs, tmpdir=None):
    from concourse import bass_utils

    nc, in_maps, cores = _prepare(**inputs)
    return bass_utils.run_bass_kernel_spmd(
        nc, in_maps, core_ids=list(range(M_CORES)), trace=True, tmpdir=tmpdir)
